# revision 10
# baseline (speedup 1.0000x reference)
"""Mixture-of-Experts (N=16384, D=1024, E=8, top-2) on 8 Trainium2 NeuronCores.

Expert-parallel sharding: core e owns expert e's [D, D] weight. The host
computes the (tiny) router decisions to derive each expert's token list; each
core gathers its assigned token rows from the full x via indirect DMA,
transposes them on the PE, runs the [tokens, D] @ W_e.T GEMM in float32r
(full-rate fp32 on the PE), adds bias and applies the renormalized top-2 gate
on-chip, and writes the scaled rows out. The host scatter-adds the two
per-token contributions back to token order.
"""

import numpy as np

import concourse.bass as bass
import concourse.mybir as mybir
from concourse.tile import TileContext
from concourse.masks import make_identity
from concourse.bass_utils import run_bass_kernel_spmd

N, D, E, TOP_K = 16384, 1024, 8, 2
P = 128
F32 = mybir.dt.float32
F32R = mybir.dt.float32r
I32 = mybir.dt.int32


def _split_multiwaits(nc, max_waits=1):
    """This walrus build accepts at most one sync wait per engine instruction.

    Hoist excess waits onto standalone EventSemaphore carriers (the same
    instruction wait_ge emits) immediately before the instruction.
    """
    for fn in nc.m.functions:
        for bb in fn.blocks:
            new = []
            for inst in bb.instructions:
                si = inst.sync_info
                if (
                    si is not None
                    and si.on_wait
                    and len(si.on_wait) > max_waits
                    and not isinstance(inst, mybir.InstEventSemaphore)
                ):
                    waits = list(si.on_wait)
                    for w in waits[:-max_waits]:
                        c = mybir.InstEventSemaphore(
                            name=f"{inst.name}-hw{w.id}", ins=[], outs=[]
                        )
                        c.engine = inst.engine
                        c.sync_info = mybir.SyncInfo(on_wait=[w], on_update=[])
                        new.append(c)
                    si.on_wait = waits[-max_waits:]
                new.append(inst)
            bb.instructions = new


def _route_host(x, context_vector, router_w, router_b, context_weight):
    logits = x @ router_w.T + router_b + context_weight[0] * context_vector
    m = logits.max(axis=1, keepdims=True)
    ex = np.exp(logits - m)
    probs = ex / ex.sum(axis=1, keepdims=True)
    # descending stable sort == jax.lax.top_k tie-breaking (lower index first)
    top2 = np.argsort(-probs, axis=1, kind="stable")[:, :TOP_K]
    p12 = np.take_along_axis(probs, top2, axis=1)
    gates = (p12 / p12.sum(axis=1, keepdims=True)).astype(np.float32)

    counts = np.bincount(top2.ravel(), minlength=E)
    avg_prob = probs.mean(axis=0)
    load_loss = np.float32(np.sum(avg_prob * (counts / N)))
    return top2, gates, counts, load_loss


def _build(C):
    nc = bass.Bass()
    x = nc.declare_dram_parameter("x", [N, D], F32, isOutput=False)
    wt = nc.declare_dram_parameter("wt", [D, D], F32, isOutput=False)  # W_e.T
    bias = nc.declare_dram_parameter("bias", [1, D], F32, isOutput=False)
    idx = nc.declare_dram_parameter("idx", [C, 1], I32, isOutput=False)
    gate = nc.declare_dram_parameter("gate", [C, 1], F32, isOutput=False)
    out = nc.declare_dram_parameter("out", [C, D], F32, isOutput=True)

    n_tiles = C // P
    KT = D // P  # 8 K-subtiles
    NC_ = D // 512  # 2 output chunks

    with TileContext(nc) as tc:
        with (
            tc.tile_pool(name="const", bufs=1) as const,
            tc.tile_pool(name="xg", bufs=3) as xgp,
            tc.tile_pool(name="xt", bufs=3) as xtp,
            tc.tile_pool(name="small", bufs=6) as smp,
            tc.tile_pool(name="outp", bufs=3) as outp,
            tc.tile_pool(name="pst", bufs=3, space="PSUM") as pst,
            tc.tile_pool(name="pso", bufs=4, space="PSUM") as pso,
        ):
            ident32 = const.tile([P, P], F32)
            make_identity(nc, ident32)
            ident = const.tile([P, P], F32R)
            nc.vector.tensor_copy(ident[:], ident32[:])
            ident_r = ident[:]

            # bias broadcast to all 128 partitions (partition-step-0 DRAM AP)
            bias_b = const.tile([P, D], F32)
            nc.scalar.dma_start(out=bias_b[:], in_=bias[:, :].to_broadcast([P, D]))

            # resident transposed expert weight: wts[k][p, o] = W.T[k*128+p, o]
            wt_v = wt[:, :].rearrange("(k p) o -> p k o", p=P).bitcast(F32R)
            wts = []
            for k in range(KT):
                wk = const.tile([P, D], F32R, name=f"wts{k}")
                nc.sync.dma_start(out=wk[:], in_=wt_v[:, k, :])
                wts.append(wk)

            for i in range(n_tiles):
                idx_t = smp.tile([P, 1], I32)
                nc.scalar.dma_start(idx_t[:], idx[P * i : P * (i + 1), :])
                gate_t = smp.tile([P, 1], F32)
                nc.scalar.dma_start(gate_t[:], gate[P * i : P * (i + 1), :])

                # gather this tile's 128 token rows from full x
                xg = xgp.tile([P, D], F32R)
                nc.gpsimd.indirect_dma_start(
                    out=xg[:],
                    out_offset=None,
                    in_=x[:, :].bitcast(F32R),
                    in_offset=bass.IndirectOffsetOnAxis(ap=idx_t[:, :1], axis=0),
                )

                # transpose to xt[p, k, t] (d = k*128+p on partitions)
                xt = xtp.tile([P, KT, P], F32R)
                for k in range(KT):
                    ps = pst.tile([P, P], F32, space="PSUM", name="pst")
                    nc.tensor.transpose(
                        ps[:].bitcast(F32R),
                        xg[:, k * P : (k + 1) * P],
                        ident_r,
                    )
                    # split copybacks between ACT and DVE to halve that path
                    if k % 2 == 0:
                        nc.scalar.copy(xt[:, k, :], ps[:])
                    else:
                        nc.vector.tensor_copy(xt[:, k, :], ps[:])

                outt = outp.tile([P, D], F32)
                for n in range(NC_):
                    po = pso.tile([P, 512], F32, space="PSUM", name="po")
                    for k in range(KT):
                        nc.tensor.matmul(
                            po[:],
                            xt[:, k, :],
                            wts[k][:, 512 * n : 512 * (n + 1)],
                            start=(k == 0),
                            stop=(k == KT - 1),
                        )
                    # out = psum + bias (PSUM -> SBUF)
                    nc.vector.tensor_add(
                        outt[:, 512 * n : 512 * (n + 1)], po[:], bias_b[:, 512 * n : 512 * (n + 1)]
                    )
                # apply renormalized top-2 gate (zero on padding rows)
                nc.vector.tensor_scalar_mul(outt[:], outt[:], gate_t[:, :1])
                nc.sync.dma_start(out[P * i : P * (i + 1), :], outt[:])

    _split_multiwaits(nc)
    return nc


def kernel(**inputs):
    x = np.ascontiguousarray(np.asarray(inputs["x"], dtype=np.float32))
    cv = np.asarray(inputs["context_vector"], dtype=np.float32)
    rw = np.asarray(inputs["router_w"], dtype=np.float32)
    rb = np.asarray(inputs["router_b"], dtype=np.float32)
    ew = np.asarray(inputs["expert_w"], dtype=np.float32)
    eb = np.asarray(inputs["expert_b"], dtype=np.float32)
    cw = np.asarray(inputs["context_weight"], dtype=np.float32)

    top2, gates, counts, load_loss = _route_host(x, cv, rw, rb, cw)

    C = max(P, int(-(-counts.max() // P) * P))  # uniform capacity, 128-multiple
    tok_lists, gate_lists = [], []
    for e in range(E):
        sel = np.nonzero(top2 == e)
        tok_lists.append(sel[0].astype(np.int32))
        gate_lists.append(gates[sel[0], sel[1]].astype(np.float32))

    nc = _build(C)

    in_maps = []
    for e in range(E):
        cnt = counts[e]
        idx_e = np.zeros((C, 1), np.int32)
        idx_e[:cnt, 0] = tok_lists[e]
        gate_e = np.zeros((C, 1), np.float32)
        gate_e[:cnt, 0] = gate_lists[e]
        in_maps.append(
            {
                "x": x,
                "wt": np.ascontiguousarray(ew[e].T),
                "bias": eb[e : e + 1],
                "idx": idx_e,
                "gate": gate_e,
            }
        )

    res = run_bass_kernel_spmd(nc, in_maps, list(range(E)))
    globals()["_last_results"] = res

    out = np.zeros((N, D), np.float32)
    for e in range(E):
        r = res.results[e]["out"]
        out[tok_lists[e]] += r[: counts[e]]
    return out, load_loss


# revision 11
# speedup vs baseline: 1.1351x; 1.1351x over previous
"""Mixture-of-Experts (N=16384, D=1024, E=8, top-2) on 8 Trainium2 NeuronCores.

Expert-parallel sharding: core e owns expert e's [D, D] weight. The host
computes the (tiny) router decisions to derive each expert's token list; each
core gathers its assigned token rows from the full x via indirect DMA,
transposes them on the PE, runs the [tokens, D] @ W_e.T GEMM in float32r
(full-rate fp32 on the PE), adds bias and applies the renormalized top-2 gate
on-chip, and writes the scaled rows out. The host scatter-adds the two
per-token contributions back to token order.
"""

import numpy as np

import concourse.bass as bass
import concourse.mybir as mybir
from concourse.tile import TileContext
from concourse.masks import make_identity
from concourse.bass_utils import run_bass_kernel_spmd

N, D, E, TOP_K = 16384, 1024, 8, 2
P = 128
F32 = mybir.dt.float32
F32R = mybir.dt.float32r
I32 = mybir.dt.int32


def _split_multiwaits(nc, max_waits=1):
    """This walrus build accepts at most one sync wait per engine instruction.

    Hoist excess waits onto standalone EventSemaphore carriers (the same
    instruction wait_ge emits) immediately before the instruction.
    """
    for fn in nc.m.functions:
        for bb in fn.blocks:
            new = []
            for inst in bb.instructions:
                si = inst.sync_info
                if (
                    si is not None
                    and si.on_wait
                    and len(si.on_wait) > max_waits
                    and not isinstance(inst, mybir.InstEventSemaphore)
                ):
                    waits = list(si.on_wait)
                    for w in waits[:-max_waits]:
                        c = mybir.InstEventSemaphore(
                            name=f"{inst.name}-hw{w.id}", ins=[], outs=[]
                        )
                        c.engine = inst.engine
                        c.sync_info = mybir.SyncInfo(on_wait=[w], on_update=[])
                        new.append(c)
                    si.on_wait = waits[-max_waits:]
                new.append(inst)
            bb.instructions = new


def _route_host(x, context_vector, router_w, router_b, context_weight):
    logits = x @ router_w.T + router_b + context_weight[0] * context_vector
    m = logits.max(axis=1, keepdims=True)
    ex = np.exp(logits - m)
    probs = ex / ex.sum(axis=1, keepdims=True)
    # descending stable sort == jax.lax.top_k tie-breaking (lower index first)
    top2 = np.argsort(-probs, axis=1, kind="stable")[:, :TOP_K]
    p12 = np.take_along_axis(probs, top2, axis=1)
    gates = (p12 / p12.sum(axis=1, keepdims=True)).astype(np.float32)

    counts = np.bincount(top2.ravel(), minlength=E)
    avg_prob = probs.mean(axis=0)
    load_loss = np.float32(np.sum(avg_prob * (counts / N)))
    return top2, gates, counts, load_loss


def _build(C):
    nc = bass.Bass()
    x = nc.declare_dram_parameter("x", [N, D], F32, isOutput=False)
    wt = nc.declare_dram_parameter("wt", [D, D], F32, isOutput=False)  # W_e.T
    bias = nc.declare_dram_parameter("bias", [1, D], F32, isOutput=False)
    idx = nc.declare_dram_parameter("idx", [C, 1], I32, isOutput=False)
    gate = nc.declare_dram_parameter("gate", [C, 1], F32, isOutput=False)
    out = nc.declare_dram_parameter("out", [C, D], F32, isOutput=True)

    n_tiles = C // P
    KT = D // P  # 8 K-subtiles
    NC_ = D // 512  # 2 output chunks

    with TileContext(nc) as tc:
        with (
            tc.tile_pool(name="const", bufs=1) as const,
            tc.tile_pool(name="xg", bufs=3) as xgp,
            tc.tile_pool(name="xt", bufs=3) as xtp,
            tc.tile_pool(name="small", bufs=6) as smp,
            tc.tile_pool(name="outp", bufs=3) as outp,
            tc.tile_pool(name="pst", bufs=3, space="PSUM") as pst,
            tc.tile_pool(name="pso", bufs=4, space="PSUM") as pso,
        ):
            def load_tile_inputs(i):
                idx_t = smp.tile([P, 1], I32, name="idx_t")
                nc.sync.dma_start(idx_t[:], idx[P * i : P * (i + 1), :])
                gate_t = smp.tile([P, 1], F32, name="gate_t")
                nc.sync.dma_start(gate_t[:], gate[P * i : P * (i + 1), :])
                xg = xgp.tile([P, D], F32R, name="xg")
                nc.gpsimd.indirect_dma_start(
                    out=xg[:],
                    out_offset=None,
                    in_=x[:, :].bitcast(F32R),
                    in_offset=bass.IndirectOffsetOnAxis(ap=idx_t[:, :1], axis=0),
                )
                return idx_t, gate_t, xg

            ident32 = const.tile([P, P], F32)
            make_identity(nc, ident32)
            ident = const.tile([P, P], F32R)
            nc.vector.tensor_copy(ident[:], ident32[:])
            ident_r = ident[:]

            # prefetch the first tiles' inputs ahead of the weight stream so
            # the PE starts transposing immediately
            PF = min(3, n_tiles)
            pref = [load_tile_inputs(i) for i in range(PF)]

            # resident transposed expert weight: wts[k][p, o] = W.T[k*128+p, o]
            wt_v = wt[:, :].rearrange("(k p) o -> p k o", p=P).bitcast(F32R)
            wts = []
            for k in range(KT):
                wk = const.tile([P, D], F32R, name=f"wts{k}")
                nc.sync.dma_start(out=wk[:], in_=wt_v[:, k, :])
                wts.append(wk)

            # bias broadcast to all 128 partitions (partition-step-0 DRAM AP)
            bias_b = const.tile([P, D], F32)
            nc.sync.dma_start(out=bias_b[:], in_=bias[:, :].to_broadcast([P, D]))

            for i in range(n_tiles):
                idx_t, gate_t, xg = pref[i] if i < PF else load_tile_inputs(i)

                # transpose to xt[p, k, t] (d = k*128+p on partitions)
                xt = xtp.tile([P, KT, P], F32R)
                for k in range(KT):
                    ps = pst.tile([P, P], F32, space="PSUM", name="pst")
                    nc.tensor.transpose(
                        ps[:].bitcast(F32R),
                        xg[:, k * P : (k + 1) * P],
                        ident_r,
                    )
                    # split copybacks between ACT and DVE to halve that path
                    if k % 2 == 0:
                        nc.scalar.copy(xt[:, k, :], ps[:])
                    else:
                        nc.vector.tensor_copy(xt[:, k, :], ps[:])

                outt = outp.tile([P, D], F32)
                for n in range(NC_):
                    po = pso.tile([P, 512], F32, space="PSUM", name="po")
                    for k in range(KT):
                        nc.tensor.matmul(
                            po[:],
                            xt[:, k, :],
                            wts[k][:, 512 * n : 512 * (n + 1)],
                            start=(k == 0),
                            stop=(k == KT - 1),
                        )
                    # out = psum + bias (PSUM -> SBUF)
                    nc.vector.tensor_add(
                        outt[:, 512 * n : 512 * (n + 1)], po[:], bias_b[:, 512 * n : 512 * (n + 1)]
                    )
                # apply renormalized top-2 gate (zero on padding rows)
                nc.vector.tensor_scalar_mul(outt[:], outt[:], gate_t[:, :1])
                nc.sync.dma_start(out[P * i : P * (i + 1), :], outt[:])

    _split_multiwaits(nc)
    return nc


def kernel(**inputs):
    x = np.ascontiguousarray(np.asarray(inputs["x"], dtype=np.float32))
    cv = np.asarray(inputs["context_vector"], dtype=np.float32)
    rw = np.asarray(inputs["router_w"], dtype=np.float32)
    rb = np.asarray(inputs["router_b"], dtype=np.float32)
    ew = np.asarray(inputs["expert_w"], dtype=np.float32)
    eb = np.asarray(inputs["expert_b"], dtype=np.float32)
    cw = np.asarray(inputs["context_weight"], dtype=np.float32)

    top2, gates, counts, load_loss = _route_host(x, cv, rw, rb, cw)

    C = max(P, int(-(-counts.max() // P) * P))  # uniform capacity, 128-multiple
    tok_lists, gate_lists = [], []
    for e in range(E):
        sel = np.nonzero(top2 == e)
        tok_lists.append(sel[0].astype(np.int32))
        gate_lists.append(gates[sel[0], sel[1]].astype(np.float32))

    nc = _build(C)

    in_maps = []
    for e in range(E):
        cnt = counts[e]
        idx_e = np.zeros((C, 1), np.int32)
        idx_e[:cnt, 0] = tok_lists[e]
        gate_e = np.zeros((C, 1), np.float32)
        gate_e[:cnt, 0] = gate_lists[e]
        in_maps.append(
            {
                "x": x,
                "wt": np.ascontiguousarray(ew[e].T),
                "bias": eb[e : e + 1],
                "idx": idx_e,
                "gate": gate_e,
            }
        )

    res = run_bass_kernel_spmd(nc, in_maps, list(range(E)))
    globals()["_last_results"] = res

    out = np.zeros((N, D), np.float32)
    for e in range(E):
        r = res.results[e]["out"]
        out[tok_lists[e]] += r[: counts[e]]
    return out, load_loss
